# revision 28
# baseline (speedup 1.0000x reference)
"""GroupedQueryAttention Trainium2 kernel (8 NeuronCores).

Problem: B=4, S=N=2048, d_model=2048, G=16 heads, d_head=128,
RoPE (rotary_dim=512) applied to query only, key-position mask,
out = (softmax(mask(QK^T/sqrt(dh))) @ V) @ Wo^T.

Sharding: mesh = 4 batches x 2 query-halves. core_id = b*2 + h.
Each core:
  - K/V projections for its 1024-channel half (8 heads, "local slots 0-7")
    over ALL keys; results stay SBUF-resident and are also staged (x z0/z1
    zero-masks) into a ReduceScatter input so each core receives exactly its
    PEER's K/V at a fixed address (rs_out) - an asymmetric pair exchange
    built from a symmetric collective,
  - Q projection (+RoPE) for its 1024-query half over ALL 16 head slots
    (per-core slot order: [my 8 heads, peer 8 heads], realized by host-side
    permutation of Wq columns / Wo rows / rope tables),
  - attention slots 0-7 (local K/V, overlaps the collective), then slots
    8-15 (peer K/V from rs_out),
  - full out-projection for its 1024 query rows. No trailing collective.

All matmul operands are bf16 (fp32 PSUM accumulation) - same PE rate as
fp32r at 512-wide but half the DMA/SBUF. Scores are computed transposed
(keys on partitions) so the key mask folds into the exp bias and P@V needs
no transposes. Softmax denominator: adjacent e-tiles pair-summed on DVE
(bf16 2x mode), then ones-matmuls at half rate accumulate in PSUM.
"""
import sys
import numpy as np

sys.path.insert(0, "/opt/trn_rl_repo")

from contextlib import ExitStack

import ml_dtypes

import concourse.bass as bass
import concourse.tile as tile
from concourse import bacc, mybir
from concourse.bass_utils import run_bass_kernel_spmd

FP32 = mybir.dt.float32
BF16 = mybir.dt.bfloat16
NP_BF16 = ml_dtypes.bfloat16

B = 4
S = 2048          # tokens per batch
SQ = 1024         # queries per core (half)
N = 2048          # keys per batch
D = 2048          # d_model
G = 16            # heads
DH = 128          # head dim
RD = 512          # rotary dim
CL = D // 2       # local channels (1024)
GL = 8            # local heads
SCALE = 1.0 / float(np.sqrt(DH))
MASK_BIAS = -30000.0

KT = D // 128     # contraction k-tiles (16)
NT = N // 128     # key tiles (16)
QC = SQ // 512    # query chunks per core (2)
QT = SQ // 128    # query 128-tiles per core (8)


def _build_program():
    nc = bacc.Bacc("TRN2", target_bir_lowering=False, debug=False, num_devices=8)

    # ---- external I/O (per-core contents differ; same shapes) ----
    xq = nc.dram_tensor("xq", [D, SQ], BF16, kind="ExternalInput").ap()   # query^T half
    xk = nc.dram_tensor("xk", [D, N], BF16, kind="ExternalInput").ap()    # key^T full
    xv = nc.dram_tensor("xv", [D, N], BF16, kind="ExternalInput").ap()    # value^T full
    wq = nc.dram_tensor("wq", [D, D], BF16, kind="ExternalInput").ap()    # Wq^T slot order
    wk = nc.dram_tensor("wk", [D, CL], BF16, kind="ExternalInput").ap()   # Wk[hs,:]^T
    wv = nc.dram_tensor("wv", [D, CL], BF16, kind="ExternalInput").ap()   # Wv[hs,:]^T
    wo = nc.dram_tensor("wo", [D, D], BF16, kind="ExternalInput").ap()    # Wo^T slot order
    cosT = nc.dram_tensor("cosT", [8 * 128, SQ], BF16, kind="ExternalInput").ap()
    sinT = nc.dram_tensor("sinT", [8 * 128, SQ], BF16, kind="ExternalInput").ap()
    biasm = nc.dram_tensor("biasm", [128, NT], FP32, kind="ExternalInput").ap()
    ones_c = nc.dram_tensor("ones_c", [128, 1], BF16, kind="ExternalInput").ap()
    ones_r = nc.dram_tensor("ones_r", [1, 128], BF16, kind="ExternalInput").ap()
    zmask = nc.dram_tensor("zmask", [128, 2], FP32, kind="ExternalInput").ap()
    out = nc.dram_tensor("out", [SQ, D], FP32, kind="ExternalOutput").ap()

    # ---- DRAM scratch ----
    # ReduceScatter pair exchange: block z is addressed to pair-rank z.
    # Each block: rows 0:1024 K^T local, rows 1024:2048 V natural (flat).
    rs_in = nc.dram_tensor("rs_in", [2 * D, N], BF16).ap()
    rs_out = nc.dram_tensor("rs_out", [D, N], BF16).ap()   # peer's [K^T; V-flat]
    ct_d = nc.dram_tensor("ct_d", [D, SQ], BF16).ap()      # context^T spill

    xq_r = xq.rearrange("(kt p) s -> kt p s", p=128)
    xk_r = xk.rearrange("(kt p) n -> kt p n", p=128)
    xv_r = xv.rearrange("(kt p) n -> p kt n", p=128)

    def rs_in_k(z):
        # K^T region of block z as [128, g, n]
        return rs_in[z * D:z * D + CL].rearrange("(g p) n -> p g n", p=128)

    def rs_in_v(z):
        # V region of block z as natural [keys, ch]
        return rs_in[z * D + CL:(z + 1) * D].rearrange("a (c b) -> (a c) b", c=2)

    with tile.TileContext(nc) as tc:
        with ExitStack() as top:
            consts = top.enter_context(tc.tile_pool(name="consts", bufs=1))
            bias_t = consts.tile([128, NT], FP32)
            ones_ct = consts.tile([128, 1], BF16)
            ones_rt = consts.tile([1, 128], BF16)
            z_t = consts.tile([128, 2], FP32)
            nc.sync.dma_start(out=bias_t, in_=biasm)
            nc.sync.dma_start(out=ones_ct, in_=ones_c)
            nc.sync.dma_start(out=ones_rt, in_=ones_r)
            nc.sync.dma_start(out=z_t, in_=zmask)

            with ExitStack() as kvq:   # local K/V + Q residents (freed before O)
                kvpool = kvq.enter_context(tc.tile_pool(name="kvloc", bufs=1))
                k_sb = kvpool.tile([128, GL, N], BF16)     # local K^T [dh, g, keys]
                v_sb = kvpool.tile([128, NT, CL], BF16)    # local V [keys, nt, ch]
                q_sb = kvpool.tile([128, G, SQ], BF16)     # all slots [dh, slot, q]

                # ---------- Phase K: K-projection (local 8 slots) ----------
                with ExitStack() as ph:
                    wpool = ph.enter_context(tc.tile_pool(name="wkpool", bufs=1))
                    xpool = ph.enter_context(tc.tile_pool(name="xkpool", bufs=5))
                    spool = ph.enter_context(tc.tile_pool(name="kstage", bufs=2))
                    pps = ph.enter_context(tc.tile_pool(name="kps", bufs=1, space="PSUM"))

                    wk_t = wpool.tile([128, KT, CL], BF16)
                    nc.sync.dma_start(
                        out=wk_t, in_=wk.rearrange("(kt p) c -> p kt c", p=128)
                    )
                    for nch in range(N // 512):
                        nsl = slice(nch * 512, (nch + 1) * 512)
                        psums = []
                        for g in range(GL):
                            psums.append(
                                pps.tile([128, 512], FP32, name=f"kp{g}", tag=f"kp{g}")
                            )
                        for kt in range(KT):
                            x_t = xpool.tile([128, 512], BF16, name="xk_t", tag="x")
                            nc.sync.dma_start(out=x_t, in_=xk_r[kt][:, nsl])
                            for g in range(GL):
                                nc.tensor.matmul(
                                    out=psums[g],
                                    lhsT=wk_t[:, kt, g * 128:(g + 1) * 128],
                                    rhs=x_t,
                                    start=(kt == 0),
                                    stop=(kt == KT - 1),
                                )
                        for g in range(GL):
                            nc.vector.tensor_copy(out=k_sb[:, g, nsl], in_=psums[g])
                        ka = spool.tile([128, GL, 512], BF16, name="ka", tag="ka")
                        kb = spool.tile([128, GL, 512], BF16, name="kb", tag="kb")
                        with nc.allow_low_precision(reason="zero-mask stage"):
                            nc.vector.tensor_scalar_mul(
                                ka, in0=k_sb[:, :, nsl], scalar1=z_t[:, 0:1]
                            )
                            nc.scalar.activation(
                                out=kb, in_=k_sb[:, :, nsl],
                                func=mybir.ActivationFunctionType.Copy,
                                scale=z_t[:, 1:2],
                            )
                        nc.sync.dma_start(out=rs_in_k(0)[:, :, nsl], in_=ka)
                        nc.sync.dma_start(out=rs_in_k(1)[:, :, nsl], in_=kb)

                # ---------- Phase V: V-projection (local 8 slots) ----------
                with ExitStack() as ph:
                    wpool = ph.enter_context(tc.tile_pool(name="wvpool", bufs=1))
                    xpool = ph.enter_context(tc.tile_pool(name="xvpool", bufs=2))
                    spool = ph.enter_context(tc.tile_pool(name="vstage", bufs=3))
                    pps = ph.enter_context(tc.tile_pool(name="vps", bufs=2, space="PSUM"))

                    wv_t = wpool.tile([128, KT, CL], BF16)
                    nc.sync.dma_start(
                        out=wv_t, in_=wv.rearrange("(kt p) c -> p kt c", p=128)
                    )
                    for nch in range(N // 512):
                        x_t = xpool.tile([128, KT, 512], BF16, name="xv_t", tag="x")
                        nc.sync.dma_start(
                            out=x_t, in_=xv_r[:, :, nch * 512:(nch + 1) * 512]
                        )
                        for j in range(4):
                            nt = nch * 4 + j
                            psums = []
                            for cc in range(2):
                                psums.append(
                                    pps.tile([128, 512], FP32, name=f"vp{cc}", tag=f"vp{cc}")
                                )
                            for kt in range(KT):
                                for cc in range(2):
                                    nc.tensor.matmul(
                                        out=psums[cc],
                                        lhsT=x_t[:, kt, j * 128:(j + 1) * 128],
                                        rhs=wv_t[:, kt, cc * 512:(cc + 1) * 512],
                                        start=(kt == 0),
                                        stop=(kt == KT - 1),
                                    )
                            for cc in range(2):
                                nc.vector.tensor_copy(
                                    out=v_sb[:, nt, cc * 512:(cc + 1) * 512],
                                    in_=psums[cc],
                                )
                            va = spool.tile([128, CL], BF16, name="va", tag="va")
                            vb = spool.tile([128, CL], BF16, name="vb", tag="vb")
                            with nc.allow_low_precision(reason="zero-mask stage"):
                                nc.vector.tensor_scalar_mul(
                                    va, in0=v_sb[:, nt, :], scalar1=z_t[:, 0:1]
                                )
                                nc.scalar.activation(
                                    out=vb, in_=v_sb[:, nt, :],
                                    func=mybir.ActivationFunctionType.Copy,
                                    scale=z_t[:, 1:2],
                                )
                            nc.sync.dma_start(
                                out=rs_in_v(0)[nt * 128:(nt + 1) * 128, :], in_=va
                            )
                            nc.sync.dma_start(
                                out=rs_in_v(1)[nt * 128:(nt + 1) * 128, :], in_=vb
                            )

                # Pair exchange: each core receives exactly its peer's K/V.
                nc.gpsimd.collective_compute(
                    "ReduceScatter",
                    mybir.AluOpType.add,
                    replica_groups=[[0, 1], [2, 3], [4, 5], [6, 7]],
                    ins=[rs_in],
                    outs=[rs_out],
                )
                # Peer K/V residents (allocated only after the V phase frees
                # its pools; loads wait on the collective and overlap Q+attn).
                kvrpool = kvq.enter_context(tc.tile_pool(name="kvrem", bufs=1))
                kr_sb = kvrpool.tile([128, GL, N], BF16)   # peer K^T
                vr_sb = kvrpool.tile([128, NT, CL], BF16)  # peer V
                # Pool-queue DMAs: they queue naturally behind the collective
                # without head-of-line blocking the SP queue (Q-phase loads).
                nc.gpsimd.dma_start(
                    out=kr_sb, in_=rs_out[0:CL].rearrange("(g p) n -> p g n", p=128)
                )
                nc.gpsimd.dma_start(
                    out=vr_sb,
                    in_=rs_out[CL:D].rearrange("a (c b) -> (a c) b", c=2)
                    .rearrange("(nt p) c -> p nt c", p=128),
                )

                # ---------- Phase Q: Q-projection + RoPE (16 slots) ----------
                with ExitStack() as ph:
                    wpool = ph.enter_context(tc.tile_pool(name="wqpool", bufs=3))
                    rpool = ph.enter_context(tc.tile_pool(name="ropepool", bufs=2))
                    xpool = ph.enter_context(tc.tile_pool(name="xqpool", bufs=3))
                    rsc = ph.enter_context(tc.tile_pool(name="ropescratch", bufs=2))
                    pps = ph.enter_context(tc.tile_pool(name="qps", bufs=1, space="PSUM"))

                    wq_r = wq.rearrange("(kt p) c -> p kt c", p=128)
                    cos_r = cosT.rearrange("(gt p) s -> p gt s", p=128)
                    sin_r = sinT.rearrange("(gt p) s -> p gt s", p=128)
                    for half in range(2):
                        for qc in range(QC):
                            ssl = slice(qc * 512, (qc + 1) * 512)
                            # rope-capable slots 0-3 / 8-11: per-(half,chunk)
                            # table tiles (real or identity per core)
                            cos_t = rpool.tile([128, 4, 512], BF16, name="cos", tag="cos")
                            sin_t = rpool.tile([128, 4, 512], BF16, name="sin", tag="sin")
                            nc.sync.dma_start(
                                out=cos_t,
                                in_=cos_r[:, half * 4:(half + 1) * 4, ssl],
                            )
                            nc.sync.dma_start(
                                out=sin_t,
                                in_=sin_r[:, half * 4:(half + 1) * 4, ssl],
                            )
                            psums = []
                            for g in range(GL):
                                psums.append(
                                    pps.tile([128, 512], FP32, name=f"qp{g}", tag=f"qp{g}")
                                )
                            for kt in range(KT):
                                x_t = xpool.tile([128, 512], BF16, name="xq_t", tag="x")
                                nc.sync.dma_start(out=x_t, in_=xq_r[kt][:, ssl])
                                wq_t = wpool.tile([128, CL], BF16, name="wq_t", tag="wq")
                                # Act queue is idle during the Q projection;
                                # keep the SP queue for xq/table streams.
                                nc.scalar.dma_start(
                                    out=wq_t,
                                    in_=wq_r[:, kt, half * CL:(half + 1) * CL],
                                )
                                for g in range(GL):
                                    nc.tensor.matmul(
                                        out=psums[g],
                                        lhsT=wq_t[:, g * 128:(g + 1) * 128],
                                        rhs=x_t,
                                        start=(kt == 0),
                                        stop=(kt == KT - 1),
                                    )
                            for g in range(GL):
                                slot = half * 8 + g
                                if g < 4:
                                    # rope'd (tables real or identity per core)
                                    sA = rsc.tile([128, 512], FP32, name="rA", tag="rA")
                                    sB = rsc.tile([128, 512], FP32, name="rB", tag="rB")
                                    nc.vector.tensor_mul(
                                        out=sA, in0=psums[g], in1=cos_t[:, g, :]
                                    )
                                    nc.vector.tensor_mul(
                                        out=sB, in0=psums[g ^ 2], in1=sin_t[:, g, :]
                                    )
                                    nc.vector.tensor_add(
                                        out=q_sb[:, slot, ssl], in0=sA, in1=sB
                                    )
                                else:
                                    nc.vector.tensor_copy(
                                        out=q_sb[:, slot, ssl], in_=psums[g]
                                    )

                # ---------- Phase A: attention, slots 0-7 then 8-15 ----------
                with ExitStack() as ph:
                    cpool = ph.enter_context(tc.tile_pool(name="cstage", bufs=3))
                    epool = ph.enter_context(tc.tile_pool(name="epool", bufs=2))
                    fpool = ph.enter_context(tc.tile_pool(name="fpool", bufs=2))
                    rpool = ph.enter_context(tc.tile_pool(name="rpool", bufs=2))
                    sps = ph.enter_context(tc.tile_pool(name="sps", bufs=2, space="PSUM"))
                    ups = ph.enter_context(tc.tile_pool(name="ups", bufs=1, space="PSUM"))
                    dps = ph.enter_context(tc.tile_pool(name="dps", bufs=1, space="PSUM"))

                    def smm(s_t, kk, g, nt, q_t):
                        # scores^T for key tile nt, all 1024 queries (2 psum
                        # banks, one matmul per bank)
                        for hh in range(2):
                            nc.tensor.matmul(
                                out=s_t[:, hh * 512:(hh + 1) * 512],
                                lhsT=kk[:, g, nt * 128:(nt + 1) * 128],
                                rhs=q_t[:, hh * 512:(hh + 1) * 512],
                                start=True, stop=True,
                            )

                    def slot_kvg(slot):
                        kk = k_sb if slot < GL else kr_sb
                        vv = v_sb if slot < GL else vr_sb
                        return kk, vv, slot % GL

                    # first-score matmul of slot 0; subsequent slots' are
                    # emitted before the previous slot's normalization tail
                    # so the Act engine never waits at slot boundaries.
                    kk0, _, g0 = slot_kvg(0)
                    s_first = sps.tile([128, SQ], FP32, name="s_ps", tag="s")
                    smm(s_first, kk0, g0, 0, q_sb[:, 0, :])

                    for slot in range(G):
                        kk, vv, g = slot_kvg(slot)
                        q_t = q_sb[:, slot, :]            # [128, 1024]
                        u_ps = ups.tile([128, SQ], FP32, name="u_ps", tag="u")
                        d_ps = [
                            dps.tile([1, 512], FP32, name="d0", tag="d0"),
                            dps.tile([1, 512], FP32, name="d1", tag="d1"),
                        ]
                        e_tiles = [None, None]
                        # DVE pair/quad adds for the softmax denominator, then
                        # quad ones-matmuls accumulating in PSUM (PE relief).
                        f_pairs = []
                        s_prev = s_first
                        for nt in range(NT):
                            e_t = epool.tile(
                                [128, SQ], BF16, name="e_t", tag=f"e{nt % 2}"
                            )
                            nc.scalar.activation(
                                out=e_t, in_=s_prev,
                                func=mybir.ActivationFunctionType.Exp,
                                bias=bias_t[:, nt:nt + 1], scale=SCALE,
                            )
                            if nt + 1 < NT:
                                s_prev = sps.tile([128, SQ], FP32, name="s_ps", tag="s")
                                smm(s_prev, kk, g, nt + 1, q_t)
                            for hh in range(2):
                                nc.tensor.matmul(
                                    out=u_ps[:, hh * 512:(hh + 1) * 512],
                                    lhsT=vv[:, nt, g * 128:(g + 1) * 128],
                                    rhs=e_t[:, hh * 512:(hh + 1) * 512],
                                    start=(nt == 0), stop=(nt == NT - 1),
                                )
                            e_tiles[nt % 2] = e_t
                            if nt % 2 == 1:
                                f_t = fpool.tile(
                                    [128, SQ], BF16, name="f_t", tag=f"f{(nt // 2) % 3}"
                                )
                                nc.vector.tensor_add(
                                    out=f_t, in0=e_tiles[0], in1=e_tiles[1]
                                )
                                f_pairs.append(f_t)
                                if len(f_pairs) == 2:
                                    # quad; d_ps accumulates over the 4 quads
                                    jj = (nt - 3) // 4
                                    qd = fpool.tile(
                                        [128, SQ], BF16, name="q_t", tag=f"q{jj % 2}"
                                    )
                                    nc.vector.tensor_add(
                                        out=qd, in0=f_pairs[0], in1=f_pairs[1]
                                    )
                                    f_pairs = []
                                    for hh in range(2):
                                        nc.tensor.matmul(
                                            out=d_ps[hh], lhsT=ones_ct,
                                            rhs=qd[:, hh * 512:(hh + 1) * 512],
                                            start=(jj == 0), stop=(jj == 3),
                                        )
                        # next slot's first scores go on the PE queue before
                        # the tail's broadcast matmuls (which wait on DVE)
                        if slot + 1 < G:
                            kk1, _, g1 = slot_kvg(slot + 1)
                            s_first = sps.tile([128, SQ], FP32, name="s_ps", tag="s")
                            smm(s_first, kk1, g1, 0, q_sb[:, slot + 1, :])
                        r_t = rpool.tile([1, SQ], BF16, name="r_t", tag="r")
                        with nc.allow_low_precision(reason="bf16 softmax scale"):
                            nc.vector.reciprocal(out=r_t[:, 0:512], in_=d_ps[0])
                            nc.vector.reciprocal(out=r_t[:, 512:1024], in_=d_ps[1])
                        # broadcast 1/d across partitions; banks shared with sps
                        b_ps = [
                            sps.tile([128, SQ], FP32, name="b_ps", tag="s")[:, 0:512],
                            sps.tile([128, SQ], FP32, name="b_ps2", tag="s")[:, 0:512],
                        ]
                        for hh in range(2):
                            nc.tensor.matmul(
                                out=b_ps[hh], lhsT=ones_rt,
                                rhs=r_t[:, hh * 512:(hh + 1) * 512],
                                start=True, stop=True,
                            )
                        b_sb = rpool.tile([128, SQ], BF16, name="b_sb", tag="bsb")
                        c_t = cpool.tile([128, SQ], BF16, name="c_t", tag="c")
                        with nc.allow_low_precision(reason="bf16 softmax scale"):
                            for hh in range(2):
                                nc.vector.tensor_copy(
                                    out=b_sb[:, hh * 512:(hh + 1) * 512], in_=b_ps[hh]
                                )
                            nc.vector.tensor_mul(out=c_t, in0=u_ps, in1=b_sb)
                        nc.sync.dma_start(
                            out=ct_d.rearrange("(g p) s -> g p s", p=128)[slot],
                            in_=c_t,
                        )

            # ---------- Phase O: out = C @ Wo^T (full, local queries) ----------
            with ExitStack() as ph:
                wpool = ph.enter_context(tc.tile_pool(name="wopool", bufs=1))
                cpool = ph.enter_context(tc.tile_pool(name="octpool", bufs=3))
                oopool = ph.enter_context(tc.tile_pool(name="oout", bufs=2))
                pps = ph.enter_context(tc.tile_pool(name="ops", bufs=2, space="PSUM"))

                wo_r = wo.rearrange("(g p) c -> p g c", p=128)
                # per-slice DMAs: the first O matmul only waits for slice 0
                wo_t = wpool.tile([128, G, D], BF16)
                for g in range(G):
                    nc.sync.dma_start(out=wo_t[:, g, :], in_=wo_r[:, g, :])

                ct_r = ct_d.rearrange("(g p) s -> p g s", p=128)
                for qt in range(QT):
                    c_sb = cpool.tile([128, G, 128], BF16, name="c_sb", tag="c_sb")
                    # Pool queue is idle after the collective; offload the
                    # context reloads there so SP only carries wo + out.
                    nc.gpsimd.dma_start(
                        out=c_sb, in_=ct_r[:, :, qt * 128:(qt + 1) * 128]
                    )
                    psums = []
                    for cc in range(4):
                        psums.append(
                            pps.tile([128, 512], FP32, name=f"op{cc}", tag=f"op{cc}")
                        )
                    for g in range(G):
                        for cc in range(4):
                            nc.tensor.matmul(
                                out=psums[cc],
                                lhsT=c_sb[:, g, :],
                                rhs=wo_t[:, g, cc * 512:(cc + 1) * 512],
                                start=(g == 0),
                                stop=(g == G - 1),
                            )
                    o_sb = oopool.tile([128, D], FP32, name="o_sb", tag="o_sb")
                    for cc in range(4):
                        nc.vector.tensor_copy(
                            out=o_sb[:, cc * 512:(cc + 1) * 512], in_=psums[cc]
                        )
                    nc.sync.dma_start(out=out[qt * 128:(qt + 1) * 128, :], in_=o_sb)

    nc.compile()
    return nc


_NC_CACHE = {}


def _get_program():
    if "nc" not in _NC_CACHE:
        _NC_CACHE["nc"] = _build_program()
    return _NC_CACHE["nc"]


def kernel(query, key, value, mask, position_ids, Wq, Wk, Wv, Wo, **kw):
    query = np.asarray(query, dtype=np.float32)
    key = np.asarray(key, dtype=np.float32)
    value = np.asarray(value, dtype=np.float32)
    mask = np.asarray(mask)
    position_ids = np.asarray(position_ids)
    Wq = np.asarray(Wq, dtype=np.float32)
    Wk = np.asarray(Wk, dtype=np.float32)
    Wv = np.asarray(Wv, dtype=np.float32)
    Wo = np.asarray(Wo, dtype=np.float32)

    # rope tables from actual position_ids (applied to query only)
    pos = position_ids.astype(np.float64)  # (S,)
    freq = np.arange(0, RD, 2, dtype=np.float64)
    inv_freq = 1.0 / (10000.0 ** (freq / RD))  # (RD/2,)
    pe = pos[:, None] * inv_freq[None, :]      # (S, RD/2)
    cos_full = np.tile(np.cos(pe), (1, 2)).T   # (512, S) fp64
    sin_full = np.tile(np.sin(pe), (1, 2)).T
    sin_signed = sin_full.copy()
    sin_signed[: RD // 2] *= -1.0              # partner sign

    ones_c = np.ones((128, 1), NP_BF16)
    ones_r = np.ones((1, 128), NP_BF16)

    in_maps = []
    for core in range(8):
        b, h = core // 2, core % 2
        hs = slice(h * CL, (h + 1) * CL)       # my channel half
        ps_ = slice((1 - h) * CL, (2 - h) * CL)  # peer channel half
        qs = slice(h * SQ, (h + 1) * SQ)       # my query half
        biasv = np.where(mask[b] == 0, np.float32(MASK_BIAS), np.float32(0.0))

        # slot order: [my 8 heads, peer 8 heads]
        wq_perm = np.concatenate([Wq[hs, :], Wq[ps_, :]], axis=0)  # (D, D) rows=slots
        wo_perm = np.concatenate([Wo[:, hs], Wo[:, ps_]], axis=1)  # cols=slot ch

        # rope-capable slots 0-3 and 8-11: real tables where the slot group
        # holds global heads 0-3 (channels 0-511), identity otherwise.
        cos_q = np.asarray(cos_full[:, qs], dtype=np.float64)  # (512, SQ)
        sin_q = np.asarray(sin_signed[:, qs], dtype=np.float64)
        ident_c = np.ones_like(cos_q)
        ident_s = np.zeros_like(sin_q)
        if h == 0:
            cos_tab = np.concatenate([cos_q, ident_c], axis=0)   # slots 0-3 real
            sin_tab = np.concatenate([sin_q, ident_s], axis=0)
        else:
            cos_tab = np.concatenate([ident_c, cos_q], axis=0)   # slots 8-11 real
            sin_tab = np.concatenate([ident_s, sin_q], axis=0)

        zm = np.zeros((128, 2), np.float32)
        zm[:, 1 - h] = 1  # my data goes to the peer's ReduceScatter block

        in_maps.append({
            "xq": np.ascontiguousarray(query[b].T[:, qs]).astype(NP_BF16),
            "xk": np.ascontiguousarray(key[b].T).astype(NP_BF16),
            "xv": np.ascontiguousarray(value[b].T).astype(NP_BF16),
            "wq": np.ascontiguousarray(wq_perm.T).astype(NP_BF16),
            "wk": np.ascontiguousarray(Wk[hs, :].T).astype(NP_BF16),
            "wv": np.ascontiguousarray(Wv[hs, :].T).astype(NP_BF16),
            "wo": np.ascontiguousarray(wo_perm.T).astype(NP_BF16),
            "cosT": np.ascontiguousarray(cos_tab).astype(NP_BF16),
            "sinT": np.ascontiguousarray(sin_tab).astype(NP_BF16),
            "biasm": np.ascontiguousarray(biasv.reshape(NT, 128).T),
            "ones_c": ones_c,
            "ones_r": ones_r,
            "zmask": zm,
        })

    nc = _get_program()
    res = run_bass_kernel_spmd(nc, in_maps, core_ids=list(range(8)))
    _NC_CACHE["last_res"] = res

    out = np.empty((B, S, D), np.float32)
    for core in range(8):
        b, h = core // 2, core % 2
        out[b][h * SQ:(h + 1) * SQ, :] = res.results[core]["out"]
    return out


# revision 54
# speedup vs baseline: 1.0672x; 1.0672x over previous
"""GroupedQueryAttention Trainium2 kernel (8 NeuronCores).

Problem: B=4, S=N=2048, d_model=2048, G=16 heads, d_head=128,
RoPE (rotary_dim=512) applied to query only, key-position mask,
out = (softmax(mask(QK^T/sqrt(dh))) @ V) @ Wo^T.

Sharding: mesh = 4 batches x 2 query-halves. core_id = b*2 + h.
Each core:
  - K/V projections for its 1024-channel half (8 heads, "local slots 0-7")
    over ALL keys; results stay SBUF-resident and are also staged (x z0/z1
    zero-masks) into a ReduceScatter input so each core receives exactly its
    PEER's K/V at a fixed address (rs_out) - an asymmetric pair exchange
    built from a symmetric collective,
  - Q projection (+RoPE) for its 1024-query half over ALL 16 head slots
    (per-core slot order: [my 8 heads, peer 8 heads], realized by host-side
    permutation of Wq columns / Wo rows / rope tables),
  - attention slots 0-7 (local K/V, overlaps the collective), then slots
    8-15 (peer K/V from rs_out),
  - full out-projection for its 1024 query rows. No trailing collective.

All matmul operands are bf16 (fp32 PSUM accumulation) - same PE rate as
fp32r at 512-wide but half the DMA/SBUF. Scores are computed transposed
(keys on partitions) so the key mask folds into the exp bias and P@V needs
no transposes. Softmax denominator: adjacent e-tiles pair-summed on DVE
(bf16 2x mode), then ones-matmuls at half rate accumulate in PSUM.
"""
import sys
import numpy as np

sys.path.insert(0, "/opt/trn_rl_repo")

from contextlib import ExitStack

import ml_dtypes

import concourse.bass as bass
import concourse.tile as tile
from concourse import bacc, mybir
from concourse.bass_utils import run_bass_kernel_spmd

FP32 = mybir.dt.float32
BF16 = mybir.dt.bfloat16
NP_BF16 = ml_dtypes.bfloat16

B = 4
S = 2048          # tokens per batch
SQ = 1024         # queries per core (half)
N = 2048          # keys per batch
D = 2048          # d_model
G = 16            # heads
DH = 128          # head dim
RD = 512          # rotary dim
CL = D // 2       # local channels (1024)
GL = 8            # local heads
SCALE = 1.0 / float(np.sqrt(DH))
MASK_BIAS = -30000.0

KT = D // 128     # contraction k-tiles (16)
NT = N // 128     # key tiles (16)
QC = SQ // 512    # query chunks per core (2)
QT = SQ // 128    # query 128-tiles per core (8)


def _build_program():
    nc = bacc.Bacc("TRN2", target_bir_lowering=False, debug=False, num_devices=8)

    # ---- external I/O (per-core contents differ; same shapes) ----
    xq = nc.dram_tensor("xq", [D, SQ], BF16, kind="ExternalInput").ap()   # query^T half
    xk = nc.dram_tensor("xk", [D, N], BF16, kind="ExternalInput").ap()    # key^T full
    xv = nc.dram_tensor("xv", [D, N], BF16, kind="ExternalInput").ap()    # value^T full
    wq = nc.dram_tensor("wq", [D, D], BF16, kind="ExternalInput").ap()    # Wq^T slot order
    wk = nc.dram_tensor("wk", [D, CL], BF16, kind="ExternalInput").ap()   # Wk[hs,:]^T
    wv = nc.dram_tensor("wv", [D, CL], BF16, kind="ExternalInput").ap()   # Wv[hs,:]^T
    wo = nc.dram_tensor("wo", [D, D], BF16, kind="ExternalInput").ap()    # Wo^T slot order
    cosT = nc.dram_tensor("cosT", [8 * 128, SQ], BF16, kind="ExternalInput").ap()
    sinT = nc.dram_tensor("sinT", [8 * 128, SQ], BF16, kind="ExternalInput").ap()
    biasm = nc.dram_tensor("biasm", [128, NT], FP32, kind="ExternalInput").ap()
    ones_c = nc.dram_tensor("ones_c", [128, 1], BF16, kind="ExternalInput").ap()
    ones_r = nc.dram_tensor("ones_r", [1, 128], BF16, kind="ExternalInput").ap()
    zmask = nc.dram_tensor("zmask", [128, 2], FP32, kind="ExternalInput").ap()
    out = nc.dram_tensor("out", [SQ, D], FP32, kind="ExternalOutput").ap()

    # ---- DRAM scratch ----
    # ReduceScatter pair exchange: block z is addressed to pair-rank z.
    # Each block: rows 0:1024 K^T local, rows 1024:2048 V natural (flat).
    rs_in = nc.dram_tensor("rs_in", [2 * D, N], BF16).ap()
    rs_out = nc.dram_tensor("rs_out", [D, N], BF16).ap()   # peer's [K^T; V-flat]
    ct_d = nc.dram_tensor("ct_d", [D, SQ], BF16).ap()      # context^T spill

    xq_r = xq.rearrange("(kt p) s -> kt p s", p=128)
    xk_r = xk.rearrange("(kt p) n -> kt p n", p=128)
    xv_r = xv.rearrange("(kt p) n -> p kt n", p=128)

    def rs_in_k(z):
        # K^T region of block z as [128, g, n]
        return rs_in[z * D:z * D + CL].rearrange("(g p) n -> p g n", p=128)

    def rs_in_v(z):
        # V region of block z as natural [keys, ch]
        return rs_in[z * D + CL:(z + 1) * D].rearrange("a (c b) -> (a c) b", c=2)

    with tile.TileContext(nc) as tc:
        with ExitStack() as top:
            consts = top.enter_context(tc.tile_pool(name="consts", bufs=1))
            bias_t = consts.tile([128, NT], FP32)
            ones_ct = consts.tile([128, 1], BF16)
            ones_rt = consts.tile([1, 128], BF16)
            z_t = consts.tile([128, 2], FP32)
            nc.sync.dma_start(out=bias_t, in_=biasm)
            nc.sync.dma_start(out=ones_ct, in_=ones_c)
            nc.sync.dma_start(out=ones_rt, in_=ones_r)
            nc.sync.dma_start(out=z_t, in_=zmask)

            with ExitStack() as kvq:   # local K/V + Q residents (freed before O)
                kvpool = kvq.enter_context(tc.tile_pool(name="kvloc", bufs=1))
                k_sb = kvpool.tile([128, GL, N], BF16)     # local K^T [dh, g, keys]
                v_sb = kvpool.tile([128, NT, CL], BF16)    # local V [keys, nt, ch]
                q_sb = kvpool.tile([128, G, SQ], BF16)     # all slots [dh, slot, q]

                # ---------- Phase K: K-projection (local 8 slots) ----------
                with ExitStack() as ph:
                    wpool = ph.enter_context(tc.tile_pool(name="wkpool", bufs=1))
                    xpool = ph.enter_context(tc.tile_pool(name="xkpool", bufs=5))
                    spool = ph.enter_context(tc.tile_pool(name="kstage", bufs=2))
                    pps = ph.enter_context(tc.tile_pool(name="kps", bufs=1, space="PSUM"))

                    wk_t = wpool.tile([128, KT, CL], BF16)
                    nc.sync.dma_start(
                        out=wk_t, in_=wk.rearrange("(kt p) c -> p kt c", p=128)
                    )
                    for nch in range(N // 512):
                        nsl = slice(nch * 512, (nch + 1) * 512)
                        psums = []
                        for g in range(GL):
                            psums.append(
                                pps.tile([128, 512], FP32, name=f"kp{g}", tag=f"kp{g}")
                            )
                        for kt in range(KT):
                            x_t = xpool.tile([128, 512], BF16, name="xk_t", tag="x")
                            nc.sync.dma_start(out=x_t, in_=xk_r[kt][:, nsl])
                            for g in range(GL):
                                nc.tensor.matmul(
                                    out=psums[g],
                                    lhsT=wk_t[:, kt, g * 128:(g + 1) * 128],
                                    rhs=x_t,
                                    start=(kt == 0),
                                    stop=(kt == KT - 1),
                                )
                        for g in range(GL):
                            nc.vector.tensor_copy(out=k_sb[:, g, nsl], in_=psums[g])
                        ka = spool.tile([128, GL, 512], BF16, name="ka", tag="ka")
                        kb = spool.tile([128, GL, 512], BF16, name="kb", tag="kb")
                        with nc.allow_low_precision(reason="zero-mask stage"):
                            nc.vector.tensor_scalar_mul(
                                ka, in0=k_sb[:, :, nsl], scalar1=z_t[:, 0:1]
                            )
                            nc.scalar.activation(
                                out=kb, in_=k_sb[:, :, nsl],
                                func=mybir.ActivationFunctionType.Copy,
                                scale=z_t[:, 1:2],
                            )
                        nc.sync.dma_start(out=rs_in_k(0)[:, :, nsl], in_=ka)
                        nc.sync.dma_start(out=rs_in_k(1)[:, :, nsl], in_=kb)

                # ---------- Phase V: V-projection (local 8 slots) ----------
                with ExitStack() as ph:
                    wpool = ph.enter_context(tc.tile_pool(name="wvpool", bufs=1))
                    xpool = ph.enter_context(tc.tile_pool(name="xvpool", bufs=2))
                    spool = ph.enter_context(tc.tile_pool(name="vstage", bufs=3))
                    pps = ph.enter_context(tc.tile_pool(name="vps", bufs=2, space="PSUM"))

                    wv_t = wpool.tile([128, KT, CL], BF16)
                    nc.sync.dma_start(
                        out=wv_t, in_=wv.rearrange("(kt p) c -> p kt c", p=128)
                    )
                    for nch in range(N // 512):
                        x_t = xpool.tile([128, KT, 512], BF16, name="xv_t", tag="x")
                        nc.sync.dma_start(
                            out=x_t, in_=xv_r[:, :, nch * 512:(nch + 1) * 512]
                        )
                        for j in range(4):
                            nt = nch * 4 + j
                            psums = []
                            for cc in range(2):
                                psums.append(
                                    pps.tile([128, 512], FP32, name=f"vp{cc}", tag=f"vp{cc}")
                                )
                            for kt in range(KT):
                                for cc in range(2):
                                    nc.tensor.matmul(
                                        out=psums[cc],
                                        lhsT=x_t[:, kt, j * 128:(j + 1) * 128],
                                        rhs=wv_t[:, kt, cc * 512:(cc + 1) * 512],
                                        start=(kt == 0),
                                        stop=(kt == KT - 1),
                                    )
                            for cc in range(2):
                                nc.vector.tensor_copy(
                                    out=v_sb[:, nt, cc * 512:(cc + 1) * 512],
                                    in_=psums[cc],
                                )
                            va = spool.tile([128, CL], BF16, name="va", tag="va")
                            vb = spool.tile([128, CL], BF16, name="vb", tag="vb")
                            with nc.allow_low_precision(reason="zero-mask stage"):
                                nc.vector.tensor_scalar_mul(
                                    va, in0=v_sb[:, nt, :], scalar1=z_t[:, 0:1]
                                )
                                nc.scalar.activation(
                                    out=vb, in_=v_sb[:, nt, :],
                                    func=mybir.ActivationFunctionType.Copy,
                                    scale=z_t[:, 1:2],
                                )
                            nc.sync.dma_start(
                                out=rs_in_v(0)[nt * 128:(nt + 1) * 128, :], in_=va
                            )
                            nc.sync.dma_start(
                                out=rs_in_v(1)[nt * 128:(nt + 1) * 128, :], in_=vb
                            )

                # Pair exchange: each core receives exactly its peer's K/V.
                nc.gpsimd.collective_compute(
                    "ReduceScatter",
                    mybir.AluOpType.add,
                    replica_groups=[[0, 1], [2, 3], [4, 5], [6, 7]],
                    ins=[rs_in],
                    outs=[rs_out],
                )
                # Peer K/V residents (allocated only after the V phase frees
                # its pools); loads are emitted AFTER the Q phase so they can
                # head-of-line wait on the collective without blocking any
                # queue that has urgent work behind them.
                kvrpool = kvq.enter_context(tc.tile_pool(name="kvrem", bufs=1))
                kr_sb = kvrpool.tile([128, GL, N], BF16)   # peer K^T
                vr_sb = kvrpool.tile([128, NT, CL], BF16)  # peer V

                # ---------- Phase Q: Q-projection + RoPE (16 slots) ----------
                with ExitStack() as ph:
                    wpool = ph.enter_context(tc.tile_pool(name="wqpool", bufs=3))
                    rpool = ph.enter_context(tc.tile_pool(name="ropepool", bufs=2))
                    xpool = ph.enter_context(tc.tile_pool(name="xqpool", bufs=3))
                    rsc = ph.enter_context(tc.tile_pool(name="ropescratch", bufs=2))
                    pps = ph.enter_context(tc.tile_pool(name="qps", bufs=1, space="PSUM"))

                    wq_r = wq.rearrange("(kt p) c -> p kt c", p=128)
                    cos_r = cosT.rearrange("(gt p) s -> p gt s", p=128)
                    sin_r = sinT.rearrange("(gt p) s -> p gt s", p=128)
                    for half in range(2):
                        for qc in range(QC):
                            ssl = slice(qc * 512, (qc + 1) * 512)
                            # rope-capable slots 0-3 / 8-11: per-(half,chunk)
                            # table tiles (real or identity per core)
                            cos_t = rpool.tile([128, 4, 512], BF16, name="cos", tag="cos")
                            sin_t = rpool.tile([128, 4, 512], BF16, name="sin", tag="sin")
                            nc.sync.dma_start(
                                out=cos_t,
                                in_=cos_r[:, half * 4:(half + 1) * 4, ssl],
                            )
                            nc.sync.dma_start(
                                out=sin_t,
                                in_=sin_r[:, half * 4:(half + 1) * 4, ssl],
                            )
                            psums = []
                            for g in range(GL):
                                psums.append(
                                    pps.tile([128, 512], FP32, name=f"qp{g}", tag=f"qp{g}")
                                )
                            for kt in range(KT):
                                x_t = xpool.tile([128, 512], BF16, name="xq_t", tag="x")
                                nc.sync.dma_start(out=x_t, in_=xq_r[kt][:, ssl])
                                wq_t = wpool.tile([128, CL], BF16, name="wq_t", tag="wq")
                                # Act queue is idle during the Q projection;
                                # keep the SP queue for xq/table streams.
                                nc.scalar.dma_start(
                                    out=wq_t,
                                    in_=wq_r[:, kt, half * CL:(half + 1) * CL],
                                )
                                for g in range(GL):
                                    nc.tensor.matmul(
                                        out=psums[g],
                                        lhsT=wq_t[:, g * 128:(g + 1) * 128],
                                        rhs=x_t,
                                        start=(kt == 0),
                                        stop=(kt == KT - 1),
                                    )
                            # last chunk's rope runs on the idle Pool engine so
                            # the DVE queue is clear when attention pair-adds
                            # start; plain copies go to the Act engine
                            ve = nc.gpsimd if (half == 1 and qc == QC - 1) else nc.vector
                            for g in range(GL):
                                slot = half * 8 + g
                                if g < 4:
                                    # rope'd (tables real or identity per core)
                                    sA = rsc.tile([128, 512], FP32, name="rA", tag="rA")
                                    sB = rsc.tile([128, 512], FP32, name="rB", tag="rB")
                                    ve.tensor_mul(
                                        out=sA, in0=psums[g], in1=cos_t[:, g, :]
                                    )
                                    ve.tensor_mul(
                                        out=sB, in0=psums[g ^ 2], in1=sin_t[:, g, :]
                                    )
                                    ve.tensor_add(
                                        out=q_sb[:, slot, ssl], in0=sA, in1=sB
                                    )
                                else:
                                    nc.scalar.activation(
                                        out=q_sb[:, slot, ssl], in_=psums[g],
                                        func=mybir.ActivationFunctionType.Copy,
                                    )

                # ---------- Phase A: attention, slots 0-7 then 8-15 ----------
                with ExitStack() as ph:
                    cpool = ph.enter_context(tc.tile_pool(name="cstage", bufs=2))
                    epool = ph.enter_context(tc.tile_pool(name="epool", bufs=3))
                    fpool = ph.enter_context(tc.tile_pool(name="fpool", bufs=1))
                    rpool = ph.enter_context(tc.tile_pool(name="rpool", bufs=2))
                    dpool = ph.enter_context(tc.tile_pool(name="dpool", bufs=2))
                    usbp = ph.enter_context(tc.tile_pool(name="usb", bufs=2))
                    sps = ph.enter_context(tc.tile_pool(name="sps", bufs=3, space="PSUM"))
                    ups = ph.enter_context(tc.tile_pool(name="ups", bufs=1, space="PSUM"))

                    def smm(s_t, kk, g, nt, q_t):
                        # scores^T for key tile nt, all 1024 queries (2 psum
                        # banks, one matmul per bank)
                        for hh in range(2):
                            nc.tensor.matmul(
                                out=s_t[:, hh * 512:(hh + 1) * 512],
                                lhsT=kk[:, g, nt * 128:(nt + 1) * 128],
                                rhs=q_t[:, hh * 512:(hh + 1) * 512],
                                start=True, stop=True,
                            )

                    def slot_kvg(slot):
                        kk = k_sb if slot < GL else kr_sb
                        vv = v_sb if slot < GL else vr_sb
                        return kk, vv, slot % GL

                    def emit_d(slot, dacc, d_t):
                        # denominator matmuls + reciprocal (inputs from the
                        # previous slot; ready, so no queue blocking)
                        r_t = rpool.tile([1, SQ], BF16, name="r_t", tag="r")
                        for hh in range(2):
                            nc.tensor.matmul(
                                out=d_t[0:1, hh * 512:(hh + 1) * 512],
                                lhsT=ones_ct,
                                rhs=dacc[:, hh * 512:(hh + 1) * 512],
                                start=True, stop=True,
                            )
                        with nc.allow_low_precision(reason="bf16 softmax scale"):
                            nc.vector.reciprocal(out=r_t, in_=d_t[0:1, :])
                        return r_t

                    def emit_b(slot, r_t, b_t):
                        for hh in range(2):
                            nc.tensor.matmul(
                                out=b_t[:, hh * 512:(hh + 1) * 512],
                                lhsT=ones_rt,
                                rhs=r_t[:, hh * 512:(hh + 1) * 512],
                                start=True, stop=True,
                            )

                    def emit_c(slot, b_sb, u_sb):
                        c_t = cpool.tile([128, SQ], BF16, name="c_t", tag="c")
                        with nc.allow_low_precision(reason="bf16 softmax scale"):
                            nc.vector.tensor_mul(out=c_t, in0=u_sb, in1=b_sb)
                        # Act-queue DMA: near-ready when emitted (tails lag a
                        # slot), so it never head-of-line blocks the exps
                        nc.scalar.dma_start(
                            out=ct_d.rearrange("(g p) s -> g p s", p=128)[slot],
                            in_=c_t,
                        )

                    pend = None       # (slot, dacc, u_sb) of previous slot
                    stage = {}
                    # first-score matmul of slot 0; subsequent slots' are
                    # emitted before the previous slot's normalization tail
                    # so the Act engine never waits at slot boundaries.
                    kk0, _, g0 = slot_kvg(0)
                    s_first = sps.tile([128, SQ], FP32, name="s_ps", tag="s")
                    smm(s_first, kk0, g0, 0, q_sb[:, 0, :])

                    for slot in range(G):
                        if slot == GL // 2:
                            # Peer K/V loads: Pool queue (idle but for the
                            # collective, which these must wait for anyway).
                            nc.gpsimd.dma_start(
                                out=kr_sb,
                                in_=rs_out[0:CL].rearrange("(g p) n -> p g n", p=128),
                            )
                            nc.gpsimd.dma_start(
                                out=vr_sb,
                                in_=rs_out[CL:D].rearrange("a (c b) -> (a c) b", c=2)
                                .rearrange("(nt p) c -> p nt c", p=128),
                            )
                        kk, vv, g = slot_kvg(slot)
                        q_t = q_sb[:, slot, :]            # [128, 1024]
                        u_ps = ups.tile([128, SQ], FP32, name="u_ps", tag="u")
                        e_tiles = [None, None]
                        acc = None
                        s_prev = s_first
                        for nt in range(NT):
                            e_t = epool.tile(
                                [128, SQ], BF16, name="e_t", tag=f"e{nt % 2}"
                            )
                            nc.scalar.activation(
                                out=e_t, in_=s_prev,
                                func=mybir.ActivationFunctionType.Exp,
                                bias=bias_t[:, nt:nt + 1], scale=SCALE,
                            )
                            if nt + 1 < NT:
                                s_prev = sps.tile([128, SQ], FP32, name="s_ps", tag="s")
                                smm(s_prev, kk, g, nt + 1, q_t)
                            elif slot + 1 < G:
                                kk1, _, g1 = slot_kvg(slot + 1)
                                s_first = sps.tile(
                                    [128, SQ], FP32, name="s_ps", tag="s"
                                )
                                smm(s_first, kk1, g1, 0, q_sb[:, slot + 1, :])
                            for hh in range(2):
                                nc.tensor.matmul(
                                    out=u_ps[:, hh * 512:(hh + 1) * 512],
                                    lhsT=vv[:, nt, g * 128:(g + 1) * 128],
                                    rhs=e_t[:, hh * 512:(hh + 1) * 512],
                                    start=(nt == 0), stop=(nt == NT - 1),
                                )
                            e_tiles[nt % 2] = e_t
                            if nt % 2 == 1:
                                f_t = fpool.tile(
                                    [128, SQ], BF16, name="f_t", tag=f"f{(nt // 2) % 2}"
                                )
                                nc.vector.tensor_add(
                                    out=f_t, in0=e_tiles[0], in1=e_tiles[1]
                                )
                                if acc is None:
                                    acc = f_t
                                else:
                                    if nt == NT - 1:
                                        nacc = dpool.tile(
                                            [128, SQ], BF16, name="dacc", tag="dacc"
                                        )
                                    else:
                                        nacc = fpool.tile(
                                            [128, SQ], BF16, name="acc",
                                            tag=f"a{(nt // 2) % 2}"
                                        )
                                    nc.vector.tensor_add(out=nacc, in0=acc, in1=f_t)
                                    acc = nacc
                            # staged tail of the PREVIOUS slot, spread so no
                            # engine queue ever blocks on a pending result;
                            # d and b share one transient PSUM tile (the
                            # reciprocal frees it before the broadcast lands)
                            if pend is not None:
                                pslot, pdacc, pusb = pend
                                if nt == 1:
                                    stage["db"] = sps.tile(
                                        [128, SQ], FP32, name="db_t", tag="s"
                                    )
                                    stage["r"] = emit_d(pslot, pdacc, stage["db"])
                                elif nt == 2:
                                    emit_b(pslot, stage["r"], stage["db"])
                                elif nt == 3:
                                    b_sb = rpool.tile(
                                        [128, SQ], BF16, name="b_sb", tag="bsb"
                                    )
                                    with nc.allow_low_precision(
                                        reason="bf16 softmax scale"
                                    ):
                                        nc.vector.tensor_copy(
                                            out=b_sb, in_=stage["db"]
                                        )
                                    stage["bsb"] = b_sb
                                elif nt == 5:
                                    emit_c(pslot, stage["bsb"], pusb)
                        # free u's PSUM bank early so the next slot never waits
                        u_sb = usbp.tile([128, SQ], BF16, name="u_sb", tag="usb")
                        with nc.allow_low_precision(reason="bf16 softmax scale"):
                            nc.vector.tensor_copy(out=u_sb, in_=u_ps)
                        pend = (slot, acc, u_sb)
                    # flush the last slot's tail
                    pslot, pdacc, pusb = pend
                    db_t = sps.tile([128, SQ], FP32, name="db_t", tag="s")
                    r_t = emit_d(pslot, pdacc, db_t)
                    emit_b(pslot, r_t, db_t)
                    b_sb = rpool.tile([128, SQ], BF16, name="b_sb", tag="bsb")
                    with nc.allow_low_precision(reason="bf16 softmax scale"):
                        nc.vector.tensor_copy(out=b_sb, in_=db_t)
                    emit_c(pslot, b_sb, pusb)

            # ---------- Phase O: out = C @ Wo^T (full, local queries) ----------
            with ExitStack() as ph:
                wpool = ph.enter_context(tc.tile_pool(name="wopool", bufs=1))
                cpool = ph.enter_context(tc.tile_pool(name="octpool", bufs=3))
                oopool = ph.enter_context(tc.tile_pool(name="oout", bufs=2))
                pps = ph.enter_context(tc.tile_pool(name="ops", bufs=2, space="PSUM"))

                wo_r = wo.rearrange("(g p) c -> p g c", p=128)
                # per-slice DMAs: the first O matmul only waits for slice 0
                wo_t = wpool.tile([128, G, D], BF16)
                for g in range(G):
                    nc.sync.dma_start(out=wo_t[:, g, :], in_=wo_r[:, g, :])

                ct_r = ct_d.rearrange("(g p) s -> p g s", p=128)
                for qt in range(QT):
                    c_sb = cpool.tile([128, G, 128], BF16, name="c_sb", tag="c_sb")
                    # Pool queue is idle after the collective; offload the
                    # context reloads there so SP only carries wo + out.
                    nc.gpsimd.dma_start(
                        out=c_sb, in_=ct_r[:, :, qt * 128:(qt + 1) * 128]
                    )
                    psums = []
                    for cc in range(4):
                        psums.append(
                            pps.tile([128, 512], FP32, name=f"op{cc}", tag=f"op{cc}")
                        )
                    for g in range(G):
                        for cc in range(4):
                            nc.tensor.matmul(
                                out=psums[cc],
                                lhsT=c_sb[:, g, :],
                                rhs=wo_t[:, g, cc * 512:(cc + 1) * 512],
                                start=(g == 0),
                                stop=(g == G - 1),
                            )
                    o_sb = oopool.tile([128, D], FP32, name="o_sb", tag="o_sb")
                    for cc in range(4):
                        nc.vector.tensor_copy(
                            out=o_sb[:, cc * 512:(cc + 1) * 512], in_=psums[cc]
                        )
                    nc.sync.dma_start(out=out[qt * 128:(qt + 1) * 128, :], in_=o_sb)

    nc.compile()
    return nc


_NC_CACHE = {}


def _get_program():
    if "nc" not in _NC_CACHE:
        _NC_CACHE["nc"] = _build_program()
    return _NC_CACHE["nc"]


def kernel(query, key, value, mask, position_ids, Wq, Wk, Wv, Wo, **kw):
    query = np.asarray(query, dtype=np.float32)
    key = np.asarray(key, dtype=np.float32)
    value = np.asarray(value, dtype=np.float32)
    mask = np.asarray(mask)
    position_ids = np.asarray(position_ids)
    Wq = np.asarray(Wq, dtype=np.float32)
    Wk = np.asarray(Wk, dtype=np.float32)
    Wv = np.asarray(Wv, dtype=np.float32)
    Wo = np.asarray(Wo, dtype=np.float32)

    # rope tables from actual position_ids (applied to query only)
    pos = position_ids.astype(np.float64)  # (S,)
    freq = np.arange(0, RD, 2, dtype=np.float64)
    inv_freq = 1.0 / (10000.0 ** (freq / RD))  # (RD/2,)
    pe = pos[:, None] * inv_freq[None, :]      # (S, RD/2)
    cos_full = np.tile(np.cos(pe), (1, 2)).T   # (512, S) fp64
    sin_full = np.tile(np.sin(pe), (1, 2)).T
    sin_signed = sin_full.copy()
    sin_signed[: RD // 2] *= -1.0              # partner sign

    ones_c = np.ones((128, 1), NP_BF16)
    ones_r = np.ones((1, 128), NP_BF16)

    in_maps = []
    for core in range(8):
        b, h = core // 2, core % 2
        hs = slice(h * CL, (h + 1) * CL)       # my channel half
        ps_ = slice((1 - h) * CL, (2 - h) * CL)  # peer channel half
        qs = slice(h * SQ, (h + 1) * SQ)       # my query half
        biasv = np.where(mask[b] == 0, np.float32(MASK_BIAS), np.float32(0.0))

        # slot order: [my 8 heads, peer 8 heads]
        wq_perm = np.concatenate([Wq[hs, :], Wq[ps_, :]], axis=0)  # (D, D) rows=slots
        wo_perm = np.concatenate([Wo[:, hs], Wo[:, ps_]], axis=1)  # cols=slot ch

        # rope-capable slots 0-3 and 8-11: real tables where the slot group
        # holds global heads 0-3 (channels 0-511), identity otherwise.
        cos_q = np.asarray(cos_full[:, qs], dtype=np.float64)  # (512, SQ)
        sin_q = np.asarray(sin_signed[:, qs], dtype=np.float64)
        ident_c = np.ones_like(cos_q)
        ident_s = np.zeros_like(sin_q)
        if h == 0:
            cos_tab = np.concatenate([cos_q, ident_c], axis=0)   # slots 0-3 real
            sin_tab = np.concatenate([sin_q, ident_s], axis=0)
        else:
            cos_tab = np.concatenate([ident_c, cos_q], axis=0)   # slots 8-11 real
            sin_tab = np.concatenate([ident_s, sin_q], axis=0)

        zm = np.zeros((128, 2), np.float32)
        zm[:, 1 - h] = 1  # my data goes to the peer's ReduceScatter block

        in_maps.append({
            "xq": np.ascontiguousarray(query[b].T[:, qs]).astype(NP_BF16),
            "xk": np.ascontiguousarray(key[b].T).astype(NP_BF16),
            "xv": np.ascontiguousarray(value[b].T).astype(NP_BF16),
            "wq": np.ascontiguousarray(wq_perm.T).astype(NP_BF16),
            "wk": np.ascontiguousarray(Wk[hs, :].T).astype(NP_BF16),
            "wv": np.ascontiguousarray(Wv[hs, :].T).astype(NP_BF16),
            "wo": np.ascontiguousarray(wo_perm.T).astype(NP_BF16),
            "cosT": np.ascontiguousarray(cos_tab).astype(NP_BF16),
            "sinT": np.ascontiguousarray(sin_tab).astype(NP_BF16),
            "biasm": np.ascontiguousarray(biasv.reshape(NT, 128).T),
            "ones_c": ones_c,
            "ones_r": ones_r,
            "zmask": zm,
        })

    nc = _get_program()
    res = run_bass_kernel_spmd(nc, in_maps, core_ids=list(range(8)))
    _NC_CACHE["last_res"] = res

    out = np.empty((B, S, D), np.float32)
    for core in range(8):
        b, h = core // 2, core % 2
        out[b][h * SQ:(h + 1) * SQ, :] = res.results[core]["out"]
    return out
